# revision 6
# baseline (speedup 1.0000x reference)
"""CLIP cross-attention (pre-LN QKV + softmax attention + bottleneck adapter)
on 8 Trainium2 NeuronCores, batch-data-parallel (1 batch element per core).

Per-core dataflow (all shapes per batch element, S=1024 tokens, H=1024):
  LN(hs), LN(ctx) in natural layout (bn_stats), gamma/beta folded into the
  projection weights on the host; PE-transpose the normalized activations to
  [H, S] layout (fp32r); QKV projections in fp32r (full PE rate at N=512,
  ~tf32 precision); scores computed transposed per head S^T = K^T.T @ Q^T
  (fp32r); exp on ScalarE (unnormalized softmax - no max subtraction needed,
  |scores| <= ~9); P^T in bf16; PV as natural-layout matmul with an appended
  ones column for the softmax row-sums; normalize via reciprocal+scale;
  attention output staged through DRAM scratch (SBUF pressure), re-loaded for
  the adapter: D^T = Wd.T @ attn^T, tanh-gelu, U = G^T.T @ [Wu;bu], residual
  add, store.
"""

import numpy as np
import ml_dtypes

import concourse.bass as bass
import concourse.tile as tile
from concourse import bacc, mybir
from concourse.bass_utils import run_bass_kernel_spmd
from concourse.masks import make_identity
from contextlib import ExitStack

F32 = mybir.dt.float32
F32R = mybir.dt.float32r
BF16 = mybir.dt.bfloat16
AF = mybir.ActivationFunctionType
ALU = mybir.AluOpType

S = 1024
H = 1024
NH = 16
HD = 64
P = 128
NCORES = 8
EPS = 1e-5


def build_program(reps=1):
    nc = bacc.Bacc("TRN2", target_bir_lowering=False, debug=False,
                   num_devices=NCORES)

    hs = nc.dram_tensor("hs", [S, H], F32R, kind="ExternalInput")
    cx = nc.dram_tensor("cx", [S, H], F32R, kind="ExternalInput")
    wq = nc.dram_tensor("wq", [H, H], F32R, kind="ExternalInput")
    wk = nc.dram_tensor("wk", [H, H], F32R, kind="ExternalInput")
    wv = nc.dram_tensor("wv", [H, H], F32R, kind="ExternalInput")
    bq = nc.dram_tensor("bq", [P, 8], F32, kind="ExternalInput")
    bk = nc.dram_tensor("bk", [P, 8], F32, kind="ExternalInput")
    bv = nc.dram_tensor("bv", [1, H], BF16, kind="ExternalInput")
    wd = nc.dram_tensor("wd", [H, HD], BF16, kind="ExternalInput")
    bd = nc.dram_tensor("bd", [HD, 1], F32, kind="ExternalInput")
    wub = nc.dram_tensor("wub", [HD + 1, H], BF16, kind="ExternalInput")
    out = nc.dram_tensor("out", [S, H], F32, kind="ExternalOutput")

    with tile.TileContext(nc) as tc, ExitStack() as ctx:
        pc = ctx.enter_context(tc.tile_pool(name="const", bufs=1))
        pbig = ctx.enter_context(tc.tile_pool(name="big", bufs=2))
        pw = ctx.enter_context(tc.tile_pool(name="w", bufs=8))
        pq = ctx.enter_context(tc.tile_pool(name="q", bufs=1))
        pk = ctx.enter_context(tc.tile_pool(name="k", bufs=1))
        pv = ctx.enter_context(tc.tile_pool(name="v", bufs=1))
        pxl = ctx.enter_context(tc.tile_pool(name="xl", bufs=4))
        pstat = ctx.enter_context(tc.tile_pool(name="stat", bufs=2))
        phst = ctx.enter_context(tc.tile_pool(name="hstrip", bufs=1))
        pout = ctx.enter_context(tc.tile_pool(name="outp", bufs=2))
        pg = ctx.enter_context(tc.tile_pool(name="g", bufs=1))
        pdram = ctx.enter_context(tc.tile_pool(name="dram", bufs=1, space="DRAM"))
        pps_t = ctx.enter_context(tc.tile_pool(name="pst", bufs=2, space="PSUM"))
        pps_m = ctx.enter_context(tc.tile_pool(name="psm", bufs=3, space="PSUM"))
        pps_o = ctx.enter_context(tc.tile_pool(name="pso", bufs=3, space="PSUM"))

        # constants / small inputs
        id0 = pc.tile([P, P], F32)
        make_identity(nc, id0[:])
        idr = pc.tile([P, P], F32R)
        nc.vector.tensor_copy(idr[:], id0[:])
        bq_sb = pc.tile([P, 8], F32)
        nc.sync.dma_start(bq_sb[:], bq[:])
        bk_sb = pc.tile([P, 8], F32)
        nc.sync.dma_start(bk_sb[:], bk[:])
        bv_sb = pc.tile([P, H], BF16)
        nc.sync.dma_start(bv_sb[:], bv[:].partition_broadcast(P)[:, 0, :])
        wd_sb = pc.tile([P, 8, HD], BF16)
        nc.sync.dma_start(wd_sb[:], wd[:].rearrange("(c p) a -> p c a", p=P))
        bd_sb = pc.tile([HD, 1], F32)
        nc.sync.dma_start(bd_sb[:], bd[:])
        wub_sb = pc.tile([HD + 1, H], BF16)
        nc.sync.dma_start(wub_sb[:], wub[:])

        eps_sb = pc.tile([P, 1], F32)
        nc.vector.memset(eps_sb[:], EPS)

        vt = pv.tile([P, 8, NH, HD + 1], BF16, tag="V")
        nc.vector.memset(vt[:, :, :, HD:HD + 1], 1.0)
        gt = pg.tile([HD + 1, H], BF16, tag="gt")
        nc.vector.memset(gt[HD:HD + 1, :], 1.0)

        attn_scr = pdram.tile([S, H], F32R, tag="scr")

        qT = pq.tile([P, 8, S], F32R, tag="qT")
        kT = pk.tile([P, 8, S], F32R, tag="kT")

        loop_ctx = ExitStack()
        if reps > 1:
            # hardware loop around the whole kernel body, for timing runs
            hints = (nc.tensor.engine, nc.vector.engine, nc.scalar.engine,
                     nc.sync.engine)
            loop_ctx.enter_context(tc.For_i(0, reps, 1, hint_engines=hints))
        ctx.enter_context(loop_ctx)

        def load_w(wdram):
            tiles = []
            for kk in range(8):
                wt = pw.tile([P, H], F32R, tag="wc")
                nc.sync.dma_start(wt[:], wdram[kk * P:(kk + 1) * P, :])
                tiles.append(wt)
            return tiles

        def ln_transpose(xdram, dstT):
            # LN in natural layout (stats over free dim), then PE-transpose
            # each 128x128 block into dstT ([H-part chunk, token] layout).
            for m in range(8):
                xt = pxl.tile([P, H], F32R, tag="xl")
                nc.sync.dma_start(xt[:], xdram[m * P:(m + 1) * P, :])
                x32 = xt[:].bitcast(F32)
                st = pstat.tile([P, 2, 6], F32, tag="st")
                nc.vector.bn_stats(st[:, 0, :], x32[:, 0:512])
                nc.vector.bn_stats(st[:, 1, :], x32[:, 512:1024])
                mv = pstat.tile([P, 2], F32, tag="mv")
                nc.vector.bn_aggr(mv[:], st[:])
                sd = pstat.tile([P, 1], F32, tag="sd")
                nc.scalar.activation(sd[:], mv[:, 1:2], AF.Sqrt, bias=eps_sb[:])
                rstd = pstat.tile([P, 1], F32, tag="rs")
                nc.vector.reciprocal(rstd[:], sd[:])
                nc.vector.tensor_scalar(xt[:], x32, mv[:, 0:1], rstd[:],
                                        ALU.subtract, ALU.mult)
                for hc in range(8):
                    pt = pps_t.tile([P, P], F32R, tag="pt")
                    nc.tensor.transpose(pt[:], xt[:, hc * P:(hc + 1) * P], idr[:])
                    nc.vector.tensor_copy(dstT[:, hc, m * P:(m + 1) * P], pt[:])

        def proj_T(wtiles, srcT, dstT, bias_sb):
            # dstT[:, m8, :] = (W.T @ src^T)[m8 chunk] + bias  (all fp32r)
            for m8 in range(8):
                for n2 in range(2):
                    pm = pps_m.tile([P, 512], F32, tag="pm")
                    for kk in range(8):
                        nc.tensor.matmul(
                            pm[:], wtiles[kk][:, m8 * P:(m8 + 1) * P],
                            srcT[:, kk, n2 * 512:(n2 + 1) * 512],
                            start=(kk == 0), stop=(kk == 7))
                    nc.vector.tensor_scalar(
                        dstT[:, m8, n2 * 512:(n2 + 1) * 512], pm[:],
                        bias_sb[:, m8:m8 + 1], None, ALU.add)

        # ---- phase 1: hs LN + transpose; Q projection
        hsT = pbig.tile([P, 8, S], F32R, tag="big")
        wq_t = load_w(wq)
        ln_transpose(hs, hsT)
        proj_T(wq_t, hsT, qT, bq_sb)

        # ---- phase 2: ctx LN + transpose; K, V projections
        ctxT = pbig.tile([P, 8, S], F32R, tag="big")
        wk_t = load_w(wk)
        ln_transpose(cx, ctxT)
        proj_T(wk_t, ctxT, kT, bk_sb)

        wv_t = load_w(wv)
        for c in range(8):
            for n2 in range(2):
                pm = pps_m.tile([P, 512], F32, tag="pm")
                for kk in range(8):
                    nc.tensor.matmul(
                        pm[:], ctxT[:, kk, c * P:(c + 1) * P],
                        wv_t[kk][:, n2 * 512:(n2 + 1) * 512],
                        start=(kk == 0), stop=(kk == 7))
                h0 = n2 * 8
                nc.vector.tensor_tensor(
                    vt[:, c, h0:h0 + 8, 0:HD], pm[:],
                    bv_sb[:, n2 * 512:(n2 + 1) * 512].rearrange(
                        "p (h c) -> p h c", c=HD),
                    ALU.add)

        # ---- phase 3: attention per head
        for h in range(NH):
            r0 = (h % 2) * HD
            hc = h // 2
            pT = pbig.tile([P, 8, S], BF16, tag="big")
            for c in range(8):
                for n2 in range(2):
                    pm = pps_m.tile([P, 512], F32, tag="pm")
                    nc.tensor.matmul(
                        pm[:], kT[r0:r0 + HD, hc, c * P:(c + 1) * P],
                        qT[r0:r0 + HD, hc, n2 * 512:(n2 + 1) * 512],
                        start=True, stop=True)
                    nc.scalar.activation(pT[:, c, n2 * 512:(n2 + 1) * 512],
                                         pm[:], AF.Exp, scale=0.125)
            hst = phst.tile([P, 8, HD], F32R, tag="hst")
            for m in range(8):
                po = pps_o.tile([P, HD + 1], F32, tag="po")
                for c in range(8):
                    nc.tensor.matmul(po[:], pT[:, c, m * P:(m + 1) * P],
                                     vt[:, c, h, :],
                                     start=(c == 0), stop=(c == 7))
                rs = pstat.tile([P, 1], F32, tag="rs2")
                nc.vector.reciprocal(rs[:], po[:, HD:HD + 1])
                nc.vector.tensor_scalar(hst[:, m, :], po[:, 0:HD], rs[:],
                                        None, ALU.mult)
            nc.sync.dma_start(
                attn_scr[:, h * HD:(h + 1) * HD].rearrange(
                    "(m p) c -> p m c", p=P), hst[:])

        # ---- phase 4: adapter + residual
        attn_T = pbig.tile([P, 8, S], BF16, tag="big")
        for n2 in range(2):
            rets = []
            for mi in range(4):
                m = n2 * 4 + mi
                rt = pxl.tile([P, H], F32R, tag="xl")
                nc.sync.dma_start(rt[:], attn_scr[m * P:(m + 1) * P, :])
                rets.append((m, rt))
            for m, rt in rets:
                for hc2 in range(8):
                    pt = pps_t.tile([P, P], F32R, tag="pt")
                    nc.tensor.transpose(pt[:], rt[:, hc2 * P:(hc2 + 1) * P],
                                        idr[:])
                    nc.vector.tensor_copy(attn_T[:, hc2, m * P:(m + 1) * P],
                                          pt[:])
            pd = pps_m.tile([P, 512], F32, tag="pm")
            for kk in range(8):
                nc.tensor.matmul(pd[0:HD, :], wd_sb[:, kk, :],
                                 attn_T[:, kk, n2 * 512:(n2 + 1) * 512],
                                 start=(kk == 0), stop=(kk == 7))
            nc.scalar.activation(gt[0:HD, n2 * 512:(n2 + 1) * 512], pd[0:HD, :],
                                 AF.Gelu_apprx_tanh, bias=bd_sb[:])
            for m, rt in rets:
                for nH in range(2):
                    pu = pps_m.tile([P, 512], F32, tag="pm")
                    nc.tensor.matmul(pu[:], gt[:, m * P:(m + 1) * P],
                                     wub_sb[:, nH * 512:(nH + 1) * 512],
                                     start=True, stop=True)
                    ot = pout.tile([P, 512], F32, tag="out")
                    nc.vector.tensor_tensor(
                        ot[:], pu[:],
                        rt[:, nH * 512:(nH + 1) * 512].bitcast(F32), ALU.add)
                    nc.sync.dma_start(
                        out[m * P:(m + 1) * P, nH * 512:(nH + 1) * 512], ot[:])

    nc.compile()
    return nc


def make_in_maps(hidden_states, context, Wq, bq, Wk, bk, Wv, bv,
                 q_gamma, q_beta, c_gamma, c_beta, Wd, bd, Wu, bu):
    f32 = np.float32
    # fold LN gamma/beta into the projection weights (host-side)
    wq_e = (q_gamma[:, None] * Wq).astype(f32)
    bq_e = (bq + q_beta @ Wq).astype(f32)
    wk_e = (c_gamma[:, None] * Wk).astype(f32)
    bk_e = (bk + c_beta @ Wk).astype(f32)
    wv_e = (c_gamma[:, None] * Wv).astype(f32)
    bv_e = (bv + c_beta @ Wv).astype(f32)

    bq_r = np.ascontiguousarray(bq_e.reshape(8, P).T)   # [P, 8]
    bk_r = np.ascontiguousarray(bk_e.reshape(8, P).T)
    bv_r = bv_e.reshape(1, H).astype(ml_dtypes.bfloat16)
    wd_b = Wd.astype(ml_dtypes.bfloat16)
    bd_r = bd.reshape(HD, 1).astype(f32)
    wub = np.vstack([Wu, bu.reshape(1, H)]).astype(ml_dtypes.bfloat16)

    shared = {
        "wq": np.ascontiguousarray(wq_e), "wk": np.ascontiguousarray(wk_e),
        "wv": np.ascontiguousarray(wv_e),
        "bq": bq_r, "bk": bk_r, "bv": bv_r,
        "wd": wd_b, "bd": bd_r, "wub": wub,
    }
    in_maps = []
    for b_ in range(NCORES):
        m = dict(shared)
        m["hs"] = np.ascontiguousarray(hidden_states[b_]).astype(f32)
        m["cx"] = np.ascontiguousarray(context[b_]).astype(f32)
        in_maps.append(m)
    return in_maps


_CACHE = {}


def get_program(reps=1):
    if reps not in _CACHE:
        _CACHE[reps] = build_program(reps=reps)
    return _CACHE[reps]


def kernel(**inputs):
    nc = get_program()
    in_maps = make_in_maps(**{k: np.asarray(v) for k, v in inputs.items()})
    res = run_bass_kernel_spmd(nc, in_maps, list(range(NCORES)))
    out = np.stack([res.results[c]["out"] for c in range(NCORES)], axis=0)
    return out.astype(np.float32)


if __name__ == "__main__":
    rng = np.random.default_rng(0)
    ins = {
        "hidden_states": rng.standard_normal((8, S, H), dtype=np.float32),
        "context": rng.standard_normal((8, S, H), dtype=np.float32),
        "Wq": rng.standard_normal((H, H), dtype=np.float32) / 32,
        "bq": np.zeros(H, np.float32),
        "Wk": rng.standard_normal((H, H), dtype=np.float32) / 32,
        "bk": np.zeros(H, np.float32),
        "Wv": rng.standard_normal((H, H), dtype=np.float32) / 32,
        "bv": np.zeros(H, np.float32),
        "q_gamma": np.ones(H, np.float32), "q_beta": np.zeros(H, np.float32),
        "c_gamma": np.ones(H, np.float32), "c_beta": np.zeros(H, np.float32),
        "Wd": rng.standard_normal((H, HD), dtype=np.float32) / 32,
        "bd": np.zeros(HD, np.float32),
        "Wu": rng.standard_normal((HD, H), dtype=np.float32) / 8,
        "bu": np.zeros(H, np.float32),
    }
    o = kernel(**ins)
    print("kernel out", o.shape, o.dtype, float(np.abs(o).mean()))


# revision 9
# speedup vs baseline: 1.2959x; 1.2959x over previous
"""CLIP cross-attention (pre-LN QKV + softmax attention + bottleneck adapter)
on 8 Trainium2 NeuronCores, batch-data-parallel (1 batch element per core).

Per-core dataflow (all shapes per batch element, S=1024 tokens, H=1024):
  LN(hs), LN(ctx) in natural layout (bn_stats), gamma/beta folded into the
  projection weights on the host; PE-transpose the normalized activations to
  [H, S] layout (fp32r); QKV projections in fp32r (full PE rate at N=512,
  ~tf32 precision); scores computed transposed per head S^T = K^T.T @ Q^T
  (fp32r); exp on ScalarE in [128,1024] batches (unnormalized softmax - no
  max subtraction needed, |scores| <= ~9); P^T in bf16; PV as natural-layout
  matmul with an appended ones column for the softmax row-sums; normalize via
  reciprocal+scale into SBUF attn tiles (reusing the weight-pool slots);
  adapter: D^T = Wd.T @ attn^T, tanh-gelu, U = G^T.T @ [Wu;bu], residual add,
  store.

Weight DMAs ride the ScalarE HWDGE queue so they are not serialized behind
the SP queue's activation-tile loads.
"""

import numpy as np
import ml_dtypes

import concourse.bass as bass
import concourse.tile as tile
from concourse import bacc, mybir
from concourse.bass_utils import run_bass_kernel_spmd
from concourse.masks import make_identity
from contextlib import ExitStack

F32 = mybir.dt.float32
F32R = mybir.dt.float32r
BF16 = mybir.dt.bfloat16
AF = mybir.ActivationFunctionType
ALU = mybir.AluOpType

S = 1024
H = 1024
NH = 16
HD = 64
P = 128
NCORES = 8
EPS = 1e-5


def build_program(reps=1):
    nc = bacc.Bacc("TRN2", target_bir_lowering=False, debug=False,
                   num_devices=NCORES)

    hs = nc.dram_tensor("hs", [S, H], F32R, kind="ExternalInput")
    cx = nc.dram_tensor("cx", [S, H], F32R, kind="ExternalInput")
    wq = nc.dram_tensor("wq", [H, H], F32R, kind="ExternalInput")
    wk = nc.dram_tensor("wk", [H, H], F32R, kind="ExternalInput")
    wv = nc.dram_tensor("wv", [H, H], F32R, kind="ExternalInput")
    bq = nc.dram_tensor("bq", [P, 8], F32, kind="ExternalInput")
    bk = nc.dram_tensor("bk", [P, 8], F32, kind="ExternalInput")
    bv = nc.dram_tensor("bv", [1, H], BF16, kind="ExternalInput")
    wd = nc.dram_tensor("wd", [H, HD], BF16, kind="ExternalInput")
    bd = nc.dram_tensor("bd", [HD, 1], F32, kind="ExternalInput")
    wub = nc.dram_tensor("wub", [HD + 1, H], BF16, kind="ExternalInput")
    out = nc.dram_tensor("out", [S, H], F32, kind="ExternalOutput")

    with tile.TileContext(nc) as tc, ExitStack() as ctx:
        pc = ctx.enter_context(tc.tile_pool(name="const", bufs=1))
        pbig = ctx.enter_context(tc.tile_pool(name="big", bufs=2))
        pw = ctx.enter_context(tc.tile_pool(name="w", bufs=9))
        pq = ctx.enter_context(tc.tile_pool(name="q", bufs=1))
        pk = ctx.enter_context(tc.tile_pool(name="k", bufs=1))
        pv = ctx.enter_context(tc.tile_pool(name="v", bufs=1))
        pxl = ctx.enter_context(tc.tile_pool(name="xl", bufs=2))
        pstat = ctx.enter_context(tc.tile_pool(name="stat", bufs=2))
        pout = ctx.enter_context(tc.tile_pool(name="outp", bufs=2))
        pg = ctx.enter_context(tc.tile_pool(name="g", bufs=1))
        pps_t = ctx.enter_context(tc.tile_pool(name="pst", bufs=2, space="PSUM"))
        pps_m = ctx.enter_context(tc.tile_pool(name="psm", bufs=2, space="PSUM"))
        pps_o = ctx.enter_context(tc.tile_pool(name="pso", bufs=2, space="PSUM"))

        # constants / small inputs
        id0 = pc.tile([P, P], F32)
        make_identity(nc, id0[:])
        idr = pc.tile([P, P], F32R)
        nc.vector.tensor_copy(idr[:], id0[:])
        bq_sb = pc.tile([P, 8], F32)
        nc.sync.dma_start(bq_sb[:], bq[:])
        bk_sb = pc.tile([P, 8], F32)
        nc.sync.dma_start(bk_sb[:], bk[:])
        bv_sb = pc.tile([P, H], BF16)
        nc.sync.dma_start(bv_sb[:], bv[:].partition_broadcast(P)[:, 0, :])
        wd_sb = pc.tile([P, 8, HD], BF16)
        nc.sync.dma_start(wd_sb[:], wd[:].rearrange("(c p) a -> p c a", p=P))
        bd_sb = pc.tile([HD, 1], F32)
        nc.sync.dma_start(bd_sb[:], bd[:])
        wub_sb = pc.tile([HD + 1, H], BF16)
        nc.sync.dma_start(wub_sb[:], wub[:])
        eps_sb = pc.tile([P, 1], F32)
        nc.vector.memset(eps_sb[:], EPS)

        vt = pv.tile([P, 8, NH, HD + 1], BF16, tag="V")
        nc.vector.memset(vt[:, :, :, HD:HD + 1], 1.0)
        gt = pg.tile([HD + 1, H], BF16, tag="gt")
        nc.vector.memset(gt[HD:HD + 1, :], 1.0)

        qT = pq.tile([P, 8, S], F32R, tag="qT")
        kT = pk.tile([P, 8, S], F32R, tag="kT")

        loop_ctx = ExitStack()
        if reps > 1:
            hints = (nc.tensor.engine, nc.vector.engine, nc.scalar.engine,
                     nc.sync.engine)
            loop_ctx.enter_context(tc.For_i(0, reps, 1, hint_engines=hints))
        ctx.enter_context(loop_ctx)

        def load_w(wdram):
            # weight chunks ride the ACT HWDGE queue (SP carries the paced
            # activation loads; a waiting DMA blocks all later SP DMAs)
            tiles = []
            for kk in range(8):
                wt = pw.tile([P, H], F32R, tag="wc")
                nc.scalar.dma_start(wt[:], wdram[kk * P:(kk + 1) * P, :])
                tiles.append(wt)
            return tiles

        def ln_transpose(xdram, dstT):
            # LN in natural layout (stats over free dim), then PE-transpose
            # 128x128 blocks, 4 at a time per PSUM tile, into dstT.
            for m in range(8):
                xt = pxl.tile([P, H], F32R, tag="xl")
                nc.sync.dma_start(xt[:], xdram[m * P:(m + 1) * P, :])
                x32 = xt[:].bitcast(F32)
                st = pstat.tile([P, 2, 6], F32, tag="st")
                nc.vector.bn_stats(st[:, 0, :], x32[:, 0:512])
                nc.vector.bn_stats(st[:, 1, :], x32[:, 512:1024])
                mv = pstat.tile([P, 2], F32, tag="mv")
                nc.vector.bn_aggr(mv[:], st[:])
                sd = pstat.tile([P, 1], F32, tag="sd")
                nc.scalar.activation(sd[:], mv[:, 1:2], AF.Sqrt, bias=eps_sb[:])
                rstd = pstat.tile([P, 1], F32, tag="rs")
                nc.vector.reciprocal(rstd[:], sd[:])
                nc.vector.tensor_scalar(xt[:], x32, mv[:, 0:1], rstd[:],
                                        ALU.subtract, ALU.mult)
                for j in range(2):
                    pt = pps_t.tile([P, 512], F32R, tag="pt")
                    for jj in range(4):
                        hc = j * 4 + jj
                        nc.tensor.transpose(pt[:, jj * P:(jj + 1) * P],
                                            xt[:, hc * P:(hc + 1) * P], idr[:])
                    nc.vector.tensor_copy(
                        dstT[:, j * 4:(j + 1) * 4, m * P:(m + 1) * P],
                        pt[:].rearrange("p (jj c) -> p jj c", c=P))

        def proj_T(wtiles, srcT, dstT, bias_sb):
            # dstT[:, m8, :] = (W.T @ src^T)[m8 chunk] + bias  (all fp32r)
            for m8 in range(8):
                pm = pps_m.tile([P, 1024], F32, tag="pm")
                for n2 in range(2):
                    for kk in range(8):
                        nc.tensor.matmul(
                            pm[:, n2 * 512:(n2 + 1) * 512],
                            wtiles[kk][:, m8 * P:(m8 + 1) * P],
                            srcT[:, kk, n2 * 512:(n2 + 1) * 512],
                            start=(kk == 0), stop=(kk == 7))
                nc.vector.tensor_scalar(dstT[:, m8, :], pm[:],
                                        bias_sb[:, m8:m8 + 1], None, ALU.add)

        # ---- phase 1: hs LN + transpose; Q projection
        hsT = pbig.tile([P, 8, S], F32R, tag="big")
        wq_t = load_w(wq)
        ln_transpose(hs, hsT)
        proj_T(wq_t, hsT, qT, bq_sb)

        # ---- phase 2: ctx LN + transpose; K, V projections
        ctxT = pbig.tile([P, 8, S], F32R, tag="big")
        wk_t = load_w(wk)
        ln_transpose(cx, ctxT)
        proj_T(wk_t, ctxT, kT, bk_sb)

        wv_t = load_w(wv)
        for c in range(8):
            pm = pps_m.tile([P, 1024], F32, tag="pm")
            for n2 in range(2):
                for kk in range(8):
                    nc.tensor.matmul(
                        pm[:, n2 * 512:(n2 + 1) * 512],
                        ctxT[:, kk, c * P:(c + 1) * P],
                        wv_t[kk][:, n2 * 512:(n2 + 1) * 512],
                        start=(kk == 0), stop=(kk == 7))
            nc.vector.tensor_tensor(
                vt[:, c, :, 0:HD], pm[:],
                bv_sb[:].rearrange("p (h c) -> p h c", c=HD), ALU.add)

        # attention output tiles reuse the (now dead) weight-pool slots
        attn = []
        for _m in range(8):
            attn_m = pw.tile([P, H], F32R, tag="wc", name=f"attn{_m}")
            attn.append(attn_m)

        # ---- phase 3: attention per head
        for h in range(NH):
            r0 = (h % 2) * HD
            hc = h // 2
            pT = pbig.tile([P, 8, S], BF16, tag="big")
            for c in range(8):
                pm = pps_m.tile([P, 1024], F32, tag="pm")
                for n2 in range(2):
                    nc.tensor.matmul(
                        pm[:, n2 * 512:(n2 + 1) * 512],
                        kT[r0:r0 + HD, hc, c * P:(c + 1) * P],
                        qT[r0:r0 + HD, hc, n2 * 512:(n2 + 1) * 512],
                        start=True, stop=True)
                nc.scalar.activation(pT[:, c, :], pm[:], AF.Exp, scale=0.125)
            for m in range(8):
                po = pps_o.tile([P, HD + 1], F32, tag="po")
                for c in range(8):
                    nc.tensor.matmul(po[:], pT[:, c, m * P:(m + 1) * P],
                                     vt[:, c, h, :],
                                     start=(c == 0), stop=(c == 7))
                rs = pstat.tile([P, 1], F32, tag="rs2")
                nc.vector.reciprocal(rs[:], po[:, HD:HD + 1])
                nc.vector.tensor_scalar(attn[m][:, h * HD:(h + 1) * HD],
                                        po[:, 0:HD], rs[:], None, ALU.mult)

        # ---- phase 4: adapter + residual
        attn_T = pbig.tile([P, 8, S], BF16, tag="big")
        for n2 in range(2):
            for mi in range(4):
                m = n2 * 4 + mi
                for j in range(2):
                    pt = pps_t.tile([P, 512], F32R, tag="pt")
                    for jj in range(4):
                        hc2 = j * 4 + jj
                        nc.tensor.transpose(pt[:, jj * P:(jj + 1) * P],
                                            attn[m][:, hc2 * P:(hc2 + 1) * P],
                                            idr[:])
                    nc.vector.tensor_copy(
                        attn_T[:, j * 4:(j + 1) * 4, m * P:(m + 1) * P],
                        pt[:].rearrange("p (jj c) -> p jj c", c=P))
            pd = pps_m.tile([P, 1024], F32, tag="pm")
            for kk in range(8):
                nc.tensor.matmul(pd[0:HD, 0:512], wd_sb[:, kk, :],
                                 attn_T[:, kk, n2 * 512:(n2 + 1) * 512],
                                 start=(kk == 0), stop=(kk == 7))
            nc.scalar.activation(gt[0:HD, n2 * 512:(n2 + 1) * 512],
                                 pd[0:HD, 0:512], AF.Gelu_apprx_tanh,
                                 bias=bd_sb[:])
            for mi in range(4):
                m = n2 * 4 + mi
                pu = pps_m.tile([P, 1024], F32, tag="pm")
                for nH in range(2):
                    nc.tensor.matmul(pu[:, nH * 512:(nH + 1) * 512],
                                     gt[:, m * P:(m + 1) * P],
                                     wub_sb[:, nH * 512:(nH + 1) * 512],
                                     start=True, stop=True)
                ot = pout.tile([P, H], F32, tag="out")
                nc.vector.tensor_tensor(ot[:], pu[:],
                                        attn[m][:].bitcast(F32), ALU.add)
                nc.sync.dma_start(out[m * P:(m + 1) * P, :], ot[:])

    nc.compile()
    return nc


def make_in_maps(hidden_states, context, Wq, bq, Wk, bk, Wv, bv,
                 q_gamma, q_beta, c_gamma, c_beta, Wd, bd, Wu, bu):
    f32 = np.float32
    # fold LN gamma/beta into the projection weights (host-side)
    wq_e = (q_gamma[:, None] * Wq).astype(f32)
    bq_e = (bq + q_beta @ Wq).astype(f32)
    wk_e = (c_gamma[:, None] * Wk).astype(f32)
    bk_e = (bk + c_beta @ Wk).astype(f32)
    wv_e = (c_gamma[:, None] * Wv).astype(f32)
    bv_e = (bv + c_beta @ Wv).astype(f32)

    bq_r = np.ascontiguousarray(bq_e.reshape(8, P).T)   # [P, 8]
    bk_r = np.ascontiguousarray(bk_e.reshape(8, P).T)
    bv_r = bv_e.reshape(1, H).astype(ml_dtypes.bfloat16)
    wd_b = Wd.astype(ml_dtypes.bfloat16)
    bd_r = bd.reshape(HD, 1).astype(f32)
    wub = np.vstack([Wu, bu.reshape(1, H)]).astype(ml_dtypes.bfloat16)

    shared = {
        "wq": np.ascontiguousarray(wq_e), "wk": np.ascontiguousarray(wk_e),
        "wv": np.ascontiguousarray(wv_e),
        "bq": bq_r, "bk": bk_r, "bv": bv_r,
        "wd": wd_b, "bd": bd_r, "wub": wub,
    }
    in_maps = []
    for b_ in range(NCORES):
        m = dict(shared)
        m["hs"] = np.ascontiguousarray(hidden_states[b_]).astype(f32)
        m["cx"] = np.ascontiguousarray(context[b_]).astype(f32)
        in_maps.append(m)
    return in_maps


_CACHE = {}


def get_program(reps=1):
    if reps not in _CACHE:
        _CACHE[reps] = build_program(reps=reps)
    return _CACHE[reps]


def kernel(**inputs):
    nc = get_program()
    in_maps = make_in_maps(**{k: np.asarray(v) for k, v in inputs.items()})
    res = run_bass_kernel_spmd(nc, in_maps, list(range(NCORES)))
    out = np.stack([res.results[c]["out"] for c in range(NCORES)], axis=0)
    return out.astype(np.float32)


# revision 21
# speedup vs baseline: 1.4087x; 1.0871x over previous
"""CLIP cross-attention (pre-LN QKV + softmax attention + bottleneck adapter)
on 8 Trainium2 NeuronCores, batch-data-parallel (1 batch element per core).

Per-core dataflow (all shapes per batch element, S=1024 tokens, H=1024):
  LN(hs), LN(ctx) in natural layout (bn_stats), gamma/beta folded into the
  projection weights on the host; PE-transpose the normalized activations to
  [H, S] layout (fp32r); QKV projections in fp32r (full PE rate at N=512,
  ~tf32 precision); scores computed transposed per head S^T = K^T.T @ Q^T
  (fp32r); exp on ScalarE in [128,1024] batches (unnormalized softmax - no
  max subtraction needed, |scores| <= ~9); P^T in bf16; PV as natural-layout
  matmul with an appended ones column for the softmax row-sums; normalize via
  reciprocal+scale into SBUF attn tiles (reusing the weight-pool slots);
  adapter: D^T = Wd.T @ attn^T, tanh-gelu, U = G^T.T @ [Wu;bu], residual add,
  store.

Weight DMAs ride the ScalarE HWDGE queue so they are not serialized behind
the SP queue's activation-tile loads.
"""

import numpy as np
import ml_dtypes

import concourse.bass as bass
import concourse.tile as tile
from concourse import bacc, mybir
from concourse.bass_utils import run_bass_kernel_spmd
from concourse.masks import make_identity
from contextlib import ExitStack

F32 = mybir.dt.float32
F32R = mybir.dt.float32r
BF16 = mybir.dt.bfloat16
AF = mybir.ActivationFunctionType
ALU = mybir.AluOpType

S = 1024
H = 1024
NH = 16
HD = 64
P = 128
NCORES = 8
EPS = 1e-5


def build_program(reps=1):
    nc = bacc.Bacc("TRN2", target_bir_lowering=False, debug=False,
                   num_devices=NCORES)

    hs = nc.dram_tensor("hs", [S, H], F32R, kind="ExternalInput")
    cx = nc.dram_tensor("cx", [S, H], F32R, kind="ExternalInput")
    wq = nc.dram_tensor("wq", [H, H], F32R, kind="ExternalInput")
    wk = nc.dram_tensor("wk", [H, H], F32R, kind="ExternalInput")
    wv = nc.dram_tensor("wv", [H, H], F32R, kind="ExternalInput")
    bq = nc.dram_tensor("bq", [P, 8], F32, kind="ExternalInput")
    bk = nc.dram_tensor("bk", [P, 8], F32, kind="ExternalInput")
    bv = nc.dram_tensor("bv", [1, H], BF16, kind="ExternalInput")
    wd = nc.dram_tensor("wd", [H, HD], BF16, kind="ExternalInput")
    bd = nc.dram_tensor("bd", [HD, 1], F32, kind="ExternalInput")
    wub = nc.dram_tensor("wub", [HD + 1, H], BF16, kind="ExternalInput")
    out = nc.dram_tensor("out", [S, H], F32, kind="ExternalOutput")

    with tile.TileContext(nc) as tc, ExitStack() as ctx:
        pc = ctx.enter_context(tc.tile_pool(name="const", bufs=1))
        pbig = ctx.enter_context(tc.tile_pool(name="big", bufs=2))
        pw = ctx.enter_context(tc.tile_pool(name="w", bufs=10))
        pq = ctx.enter_context(tc.tile_pool(name="q", bufs=1))
        pk = ctx.enter_context(tc.tile_pool(name="k", bufs=1))
        pv = ctx.enter_context(tc.tile_pool(name="v", bufs=1))
        pxl = ctx.enter_context(tc.tile_pool(name="xl", bufs=2))
        pstat = ctx.enter_context(tc.tile_pool(name="stat", bufs=2))
        pout = ctx.enter_context(tc.tile_pool(name="outp", bufs=2))
        pg = ctx.enter_context(tc.tile_pool(name="g", bufs=1))
        pps_t = ctx.enter_context(tc.tile_pool(name="pst", bufs=2, space="PSUM"))
        pps_m = ctx.enter_context(tc.tile_pool(name="psm", bufs=2, space="PSUM"))
        pps_o = ctx.enter_context(tc.tile_pool(name="pso", bufs=2, space="PSUM"))

        # constants / small inputs
        id0 = pc.tile([P, P], F32)
        make_identity(nc, id0[:])
        idr = pc.tile([P, P], F32R)
        nc.vector.tensor_copy(idr[:], id0[:])
        bq_sb = pc.tile([P, 8], F32)
        nc.sync.dma_start(bq_sb[:], bq[:])
        bk_sb = pc.tile([P, 8], F32)
        nc.sync.dma_start(bk_sb[:], bk[:])
        bv_sb = pc.tile([P, H], BF16)
        nc.sync.dma_start(bv_sb[:], bv[:].partition_broadcast(P)[:, 0, :])
        wd_sb = pc.tile([P, 8, HD], BF16)
        nc.sync.dma_start(wd_sb[:], wd[:].rearrange("(c p) a -> p c a", p=P))
        bd_sb = pc.tile([HD, 1], F32)
        nc.sync.dma_start(bd_sb[:], bd[:])
        wub_sb = pc.tile([HD + 1, H], BF16)
        nc.sync.dma_start(wub_sb[:], wub[:])
        eps_sb = pc.tile([P, 1], F32)
        nc.vector.memset(eps_sb[:], EPS)

        vt = pv.tile([P, 8, NH, HD + 1], BF16, tag="V")
        nc.vector.memset(vt[:, :, :, HD:HD + 1], 1.0)
        gt = pg.tile([HD + 1, H], BF16, tag="gt")
        nc.vector.memset(gt[HD:HD + 1, :], 1.0)

        qT = pq.tile([P, 8, S], F32R, tag="qT")
        kT = pk.tile([P, 8, S], F32R, tag="kT")

        loop_ctx = ExitStack()
        if reps > 1:
            hints = (nc.tensor.engine, nc.vector.engine, nc.scalar.engine,
                     nc.sync.engine)
            loop_ctx.enter_context(tc.For_i(0, reps, 1, hint_engines=hints))
        ctx.enter_context(loop_ctx)

        # warm the Sqrt ACT table while the first DMAs are in flight
        warm = pc.tile([P, 1], F32)
        nc.scalar.activation(warm[:], eps_sb[:], AF.Sqrt, bias=eps_sb[:])

        def load_w(wdram, eng=None):
            # weight chunks default to the GPSIMD SWDGE queue: a slot-waiting
            # DMA must not block the SP/ACT queues that pace LN and exp
            eng = eng or nc.gpsimd
            tiles = []
            for kk in range(8):
                wt = pw.tile([P, H], F32R, tag="wc")
                eng.dma_start(wt[:], wdram[kk * P:(kk + 1) * P, :])
                tiles.append(wt)
            return tiles

        def ln_transpose(xdram, dstT):
            # LN in natural layout (stats over free dim), then PE-transpose
            # 128x128 blocks, 4 at a time per PSUM tile, into dstT.
            for m in range(8):
                xt = pxl.tile([P, H], F32R, tag="xl")
                nc.sync.dma_start(xt[:], xdram[m * P:(m + 1) * P, :])
                x32 = xt[:].bitcast(F32)
                st = pstat.tile([P, 2, 6], F32, tag="st")
                nc.vector.bn_stats(st[:, 0, :], x32[:, 0:512])
                nc.vector.bn_stats(st[:, 1, :], x32[:, 512:1024])
                mv = pstat.tile([P, 2], F32, tag="mv")
                nc.vector.bn_aggr(mv[:], st[:])
                sd = pstat.tile([P, 1], F32, tag="sd")
                nc.scalar.activation(sd[:], mv[:, 1:2], AF.Sqrt, bias=eps_sb[:])
                rstd = pstat.tile([P, 1], F32, tag="rs")
                nc.vector.reciprocal(rstd[:], sd[:])
                nc.vector.tensor_scalar(xt[:], x32, mv[:, 0:1], rstd[:],
                                        ALU.subtract, ALU.mult)
                for j in range(2):
                    pt = pps_t.tile([P, 512], F32R, tag="pt")
                    for jj in range(4):
                        hc = j * 4 + jj
                        nc.tensor.transpose(pt[:, jj * P:(jj + 1) * P],
                                            xt[:, hc * P:(hc + 1) * P], idr[:])
                    # evict on ScalarE - DVE paces LN, ACT is idle here
                    nc.scalar.copy(
                        dstT[:, j * 4:(j + 1) * 4, m * P:(m + 1) * P],
                        pt[:].rearrange("p (jj c) -> p jj c", c=P))

        def proj_T(wtiles, srcT, dstT, bias_sb):
            # dstT[:, m8, :] = (W.T @ src^T)[m8 chunk] + bias  (all fp32r)
            # evicted per 512-half so the first matmuls only need half the
            # transposed source
            for m8 in range(8):
                pm = pps_m.tile([P, 1024], F32, tag="pm")
                for n2 in range(2):
                    for kk in range(8):
                        nc.tensor.matmul(
                            pm[:, n2 * 512:(n2 + 1) * 512],
                            wtiles[kk][:, m8 * P:(m8 + 1) * P],
                            srcT[:, kk, n2 * 512:(n2 + 1) * 512],
                            start=(kk == 0), stop=(kk == 7))
                    nc.scalar.activation(
                        dstT[:, m8, n2 * 512:(n2 + 1) * 512],
                        pm[:, n2 * 512:(n2 + 1) * 512], AF.Identity,
                        bias=bias_sb[:, m8:m8 + 1])

        # ---- phase 1: hs LN + transpose; Q projection
        hsT = pbig.tile([P, 8, S], F32R, tag="big")
        wq_t = load_w(wq)
        ln_transpose(hs, hsT)
        proj_T(wq_t, hsT, qT, bq_sb)

        # ---- phase 2: ctx LN + transpose; K, V projections
        ctxT = pbig.tile([P, 8, S], F32R, tag="big")
        ln_transpose(cx, ctxT)
        # Wk rides SP behind the (already issued) ctx loads - HWDGE is
        # faster than SWDGE once the slots free up at Q-proj end
        wk_t = load_w(wk, eng=nc.sync)
        proj_T(wk_t, ctxT, kT, bk_sb)

        wv_t = load_w(wv)
        for c in range(8):
            pm = pps_m.tile([P, 1024], F32, tag="pm")
            for n2 in range(2):
                for kk in range(8):
                    nc.tensor.matmul(
                        pm[:, n2 * 512:(n2 + 1) * 512],
                        ctxT[:, kk, c * P:(c + 1) * P],
                        wv_t[kk][:, n2 * 512:(n2 + 1) * 512],
                        start=(kk == 0), stop=(kk == 7))
            nc.vector.tensor_tensor(
                vt[:, c, :, 0:HD], pm[:],
                bv_sb[:].rearrange("p (h c) -> p h c", c=HD), ALU.add)

        # attention output tiles reuse the (now dead) weight-pool slots
        attn = []
        for _m in range(8):
            attn_m = pw.tile([P, H], F32R, tag="wc", name=f"attn{_m}")
            attn.append(attn_m)

        # ---- phase 3: attention.  Software-pipelined: scores+exp for head
        # h+1 are interleaved with PV of head h so the (in-order) PE always
        # has ready matmuls while ScalarE works through the exps.
        pT_tiles = {}

        def scores_chunk(h, c):
            r0 = (h % 2) * HD
            hc = h // 2
            pT = pT_tiles[h]
            pm = pps_m.tile([P, 1024], F32, tag="pm")
            for n2 in range(2):
                nc.tensor.matmul(
                    pm[:, n2 * 512:(n2 + 1) * 512],
                    kT[r0:r0 + HD, hc, c * P:(c + 1) * P],
                    qT[r0:r0 + HD, hc, n2 * 512:(n2 + 1) * 512],
                    start=True, stop=True)
            nc.scalar.activation(pT[:, c, :], pm[:], AF.Exp, scale=0.125)

        def pv_m(h, m):
            pT = pT_tiles[h]
            po = pps_o.tile([P, HD + 1], F32, tag="po")
            for c in range(8):
                nc.tensor.matmul(po[:], pT[:, c, m * P:(m + 1) * P],
                                 vt[:, c, h, :],
                                 start=(c == 0), stop=(c == 7))
            rs = pstat.tile([P, 1], F32, tag="rs2")
            nc.vector.reciprocal(rs[:], po[:, HD:HD + 1])
            nc.vector.tensor_scalar(attn[m][:, h * HD:(h + 1) * HD],
                                    po[:, 0:HD], rs[:], None, ALU.mult)

        pT_tiles[0] = pbig.tile([P, 8, S], BF16, tag="big", name="pT0")
        for c in range(8):
            scores_chunk(0, c)
        for h in range(NH):
            if h + 1 < NH:
                pT_tiles[h + 1] = pbig.tile([P, 8, S], BF16, tag="big",
                                            name=f"pT{h+1}")
            for c in range(8):
                if h + 1 < NH:
                    scores_chunk(h + 1, c)
                pv_m(h, c)
        del pT_tiles

        # ---- phase 4: adapter + residual
        attn_T = pbig.tile([P, 8, S], BF16, tag="big")
        for n2 in range(2):
            for mi in range(4):
                m = n2 * 4 + mi
                for j in range(2):
                    pt = pps_t.tile([P, 512], F32R, tag="pt")
                    for jj in range(4):
                        hc2 = j * 4 + jj
                        nc.tensor.transpose(pt[:, jj * P:(jj + 1) * P],
                                            attn[m][:, hc2 * P:(hc2 + 1) * P],
                                            idr[:])
                    nc.scalar.copy(
                        attn_T[:, j * 4:(j + 1) * 4, m * P:(m + 1) * P],
                        pt[:].rearrange("p (jj c) -> p jj c", c=P))
            pd = pps_m.tile([P, 1024], F32, tag="pm")
            for kk in range(8):
                nc.tensor.matmul(pd[0:HD, 0:512], wd_sb[:, kk, :],
                                 attn_T[:, kk, n2 * 512:(n2 + 1) * 512],
                                 start=(kk == 0), stop=(kk == 7))
            nc.scalar.activation(gt[0:HD, n2 * 512:(n2 + 1) * 512],
                                 pd[0:HD, 0:512], AF.Gelu_apprx_tanh,
                                 bias=bd_sb[:])
            for mi in range(4):
                m = n2 * 4 + mi
                pu = pps_m.tile([P, 1024], F32, tag="pm")
                for nH in range(2):
                    nc.tensor.matmul(pu[:, nH * 512:(nH + 1) * 512],
                                     gt[:, m * P:(m + 1) * P],
                                     wub_sb[:, nH * 512:(nH + 1) * 512],
                                     start=True, stop=True)
                    ot = pout.tile([P, 512], F32, tag="out")
                    nc.vector.tensor_tensor(
                        ot[:], pu[:, nH * 512:(nH + 1) * 512],
                        attn[m][:, nH * 512:(nH + 1) * 512].bitcast(F32),
                        ALU.add)
                    nc.sync.dma_start(
                        out[m * P:(m + 1) * P, nH * 512:(nH + 1) * 512],
                        ot[:])

    nc.compile()
    return nc


def make_in_maps(hidden_states, context, Wq, bq, Wk, bk, Wv, bv,
                 q_gamma, q_beta, c_gamma, c_beta, Wd, bd, Wu, bu):
    f32 = np.float32
    # fold LN gamma/beta into the projection weights (host-side)
    wq_e = (q_gamma[:, None] * Wq).astype(f32)
    bq_e = (bq + q_beta @ Wq).astype(f32)
    wk_e = (c_gamma[:, None] * Wk).astype(f32)
    bk_e = (bk + c_beta @ Wk).astype(f32)
    wv_e = (c_gamma[:, None] * Wv).astype(f32)
    bv_e = (bv + c_beta @ Wv).astype(f32)

    bq_r = np.ascontiguousarray(bq_e.reshape(8, P).T)   # [P, 8]
    bk_r = np.ascontiguousarray(bk_e.reshape(8, P).T)
    bv_r = bv_e.reshape(1, H).astype(ml_dtypes.bfloat16)
    wd_b = Wd.astype(ml_dtypes.bfloat16)
    bd_r = bd.reshape(HD, 1).astype(f32)
    wub = np.vstack([Wu, bu.reshape(1, H)]).astype(ml_dtypes.bfloat16)

    shared = {
        "wq": np.ascontiguousarray(wq_e), "wk": np.ascontiguousarray(wk_e),
        "wv": np.ascontiguousarray(wv_e),
        "bq": bq_r, "bk": bk_r, "bv": bv_r,
        "wd": wd_b, "bd": bd_r, "wub": wub,
    }
    in_maps = []
    for b_ in range(NCORES):
        m = dict(shared)
        m["hs"] = np.ascontiguousarray(hidden_states[b_]).astype(f32)
        m["cx"] = np.ascontiguousarray(context[b_]).astype(f32)
        in_maps.append(m)
    return in_maps


_CACHE = {}


def get_program(reps=1):
    if reps not in _CACHE:
        _CACHE[reps] = build_program(reps=reps)
    return _CACHE[reps]


def kernel(**inputs):
    nc = get_program()
    in_maps = make_in_maps(**{k: np.asarray(v) for k, v in inputs.items()})
    res = run_bass_kernel_spmd(nc, in_maps, list(range(NCORES)))
    out = np.stack([res.results[c]["out"] for c in range(NCORES)], axis=0)
    return out.astype(np.float32)


# revision 32
# speedup vs baseline: 1.9460x; 1.3814x over previous
"""CLIP cross-attention (pre-LN QKV + softmax attention + bottleneck adapter)
on 8 Trainium2 NeuronCores, batch-data-parallel (1 batch element per core).

Per-core dataflow (S=1024 tokens, H=1024, 16 heads x 64):
  LN in natural layout (bn_stats on DVE, apply on ScalarE, gamma/beta folded
  into the projection weights on the host), PE-transposed (fp32r) into bf16
  [H, S] activations.  QKV projections run in bf16.  Scores are computed
  transposed per head (S^T = K^T.T @ Q^T) so the softmax denominator falls
  out of the PV matmul via an appended ones column in V; exp on ScalarE in
  [128,1024] batches, no max-subtraction (|scores| <= ~9).  PV emits
  natural-layout attention rows (fp32r, normalized via reciprocal+scale on
  DVE); adapter D^T = Wd.T @ attn^T, tanh-gelu, U = G^T.T @ [Wu;bu],
  residual add, store.

Schedule: the ScalarE exp stream (~133us for 16.8M scores) is the global
pacer, so everything is arranged to start it early and never starve it:
ctx-LN -> hs-LN -> V -> per-m8 {K(m8), Q(m8), heads 2*m8, 2*m8+1} with
scores of head h interleaved against PV of head h-1 (the PE queue is
in-order, so the interleave keeps ready matmuls in front of it).  Weight
chunks are bf16 (2 KB/partition) so Wk and Wq fit in SBUF together; weight
and constant DMAs ride the GPSIMD SWDGE queue so slot-waiting loads never
block the SP/ACT queues.
"""

import numpy as np
import ml_dtypes

import concourse.bass as bass
import concourse.tile as tile
from concourse import bacc, mybir
from concourse.bass_utils import run_bass_kernel_spmd
from concourse.masks import make_identity
from contextlib import ExitStack

F32 = mybir.dt.float32
F32R = mybir.dt.float32r
BF16 = mybir.dt.bfloat16
AF = mybir.ActivationFunctionType
ALU = mybir.AluOpType

S = 1024
H = 1024
NH = 16
HD = 64
P = 128
NCORES = 8
EPS = 1e-5


def build_program(reps=1):
    nc = bacc.Bacc("TRN2", target_bir_lowering=False, debug=False,
                   num_devices=NCORES)

    hs = nc.dram_tensor("hs", [S, H], F32R, kind="ExternalInput")
    cx = nc.dram_tensor("cx", [S, H], F32R, kind="ExternalInput")
    wq = nc.dram_tensor("wq", [H, H], BF16, kind="ExternalInput")
    wk = nc.dram_tensor("wk", [H, H], BF16, kind="ExternalInput")
    wv = nc.dram_tensor("wv", [H, H], BF16, kind="ExternalInput")
    bq = nc.dram_tensor("bq", [P, 8], F32, kind="ExternalInput")
    bk = nc.dram_tensor("bk", [P, 8], F32, kind="ExternalInput")
    bv = nc.dram_tensor("bv", [1, H], BF16, kind="ExternalInput")
    wd = nc.dram_tensor("wd", [H, HD], BF16, kind="ExternalInput")
    bd = nc.dram_tensor("bd", [HD, 1], F32, kind="ExternalInput")
    wub = nc.dram_tensor("wub", [HD + 1, H], BF16, kind="ExternalInput")
    out = nc.dram_tensor("out", [S, H], F32, kind="ExternalOutput")

    with tile.TileContext(nc) as tc, ExitStack() as ctx:
        pc = ctx.enter_context(tc.tile_pool(name="const", bufs=1))
        pbig = ctx.enter_context(tc.tile_pool(name="big", bufs=2))
        pat = ctx.enter_context(tc.tile_pool(name="at", bufs=1))
        ppt = ctx.enter_context(tc.tile_pool(name="ptile", bufs=2))
        pw = ctx.enter_context(tc.tile_pool(name="w", bufs=18))
        pq = ctx.enter_context(tc.tile_pool(name="q", bufs=1))
        pk = ctx.enter_context(tc.tile_pool(name="k", bufs=1))
        pv = ctx.enter_context(tc.tile_pool(name="v", bufs=1))
        pxl = ctx.enter_context(tc.tile_pool(name="xl", bufs=2))
        pstat = ctx.enter_context(tc.tile_pool(name="stat", bufs=2))
        pout = ctx.enter_context(tc.tile_pool(name="outp", bufs=2))
        pg = ctx.enter_context(tc.tile_pool(name="g", bufs=1))
        pps_t = ctx.enter_context(tc.tile_pool(name="pst", bufs=2, space="PSUM"))
        pps_m = ctx.enter_context(tc.tile_pool(name="psm", bufs=2, space="PSUM"))
        pps_o = ctx.enter_context(tc.tile_pool(name="pso", bufs=2, space="PSUM"))

        # constants / small inputs (GPSIMD so SP starts on ctx tiles at t=0)
        id0 = pc.tile([P, P], F32)
        make_identity(nc, id0[:])
        idr = pc.tile([P, P], F32R)
        nc.vector.tensor_copy(idr[:], id0[:])
        bq_sb = pc.tile([P, 8], F32)
        nc.gpsimd.dma_start(bq_sb[:], bq[:])
        bk_sb = pc.tile([P, 8], F32)
        nc.gpsimd.dma_start(bk_sb[:], bk[:])
        bv_sb = pc.tile([P, H], BF16)
        nc.gpsimd.dma_start(bv_sb[:], bv[:].partition_broadcast(P)[:, 0, :])
        wd_sb = pc.tile([P, 8, HD], BF16)
        nc.gpsimd.dma_start(wd_sb[:], wd[:].rearrange("(c p) a -> p c a", p=P))
        bd_sb = pc.tile([HD, 1], F32)
        nc.gpsimd.dma_start(bd_sb[:], bd[:])
        wub_sb = pc.tile([HD + 1, H], BF16)
        nc.gpsimd.dma_start(wub_sb[:], wub[:])
        eps_sb = pc.tile([P, 1], F32)
        nc.vector.memset(eps_sb[:], EPS)

        vt = pv.tile([P, 8, NH, HD + 1], BF16, tag="V")
        nc.vector.memset(vt[:, :, :, HD:HD + 1], 1.0)
        gt = pg.tile([HD + 1, H], BF16, tag="gt")
        nc.vector.memset(gt[HD:HD + 1, :], 1.0)

        qT = pq.tile([P, 8, S], BF16, tag="qT")
        kT = pk.tile([P, 8, S], BF16, tag="kT")

        loop_ctx = ExitStack()
        if reps > 1:
            hints = (nc.tensor.engine, nc.vector.engine, nc.scalar.engine,
                     nc.sync.engine)
            loop_ctx.enter_context(tc.For_i(0, reps, 1, hint_engines=hints))
        ctx.enter_context(loop_ctx)

        # warm the Sqrt ACT table while the first DMAs are in flight
        warm = pc.tile([P, 1], F32)
        nc.scalar.activation(warm[:], eps_sb[:], AF.Sqrt, bias=eps_sb[:])

        def load_w(wdram):
            tiles = []
            for kk in range(8):
                wt = pw.tile([P, H], BF16, tag="wc")
                nc.gpsimd.dma_start(wt[:], wdram[kk * P:(kk + 1) * P, :])
                tiles.append(wt)
            return tiles

        def ln_transpose(xdram, dstT):
            # LN in natural layout: stats on DVE, apply on ScalarE; then
            # PE-transpose 128x128 blocks (fp32r), 4 per PSUM tile, evicted
            # to bf16 dstT on ScalarE.
            for m in range(8):
                xt = pxl.tile([P, H], F32R, tag="xl")
                nc.sync.dma_start(xt[:], xdram[m * P:(m + 1) * P, :])
                x32 = xt[:].bitcast(F32)
                st = pstat.tile([P, 2, 6], F32, tag="st")
                nc.vector.bn_stats(st[:, 0, :], x32[:, 0:512])
                nc.vector.bn_stats(st[:, 1, :], x32[:, 512:1024])
                mv = pstat.tile([P, 2], F32, tag="mv")
                nc.vector.bn_aggr(mv[:], st[:])
                sd = pstat.tile([P, 1], F32, tag="sd")
                nc.scalar.activation(sd[:], mv[:, 1:2], AF.Sqrt, bias=eps_sb[:])
                rstd = pstat.tile([P, 1], F32, tag="rs")
                nc.vector.reciprocal(rstd[:], sd[:])
                nmr = pstat.tile([P, 1], F32, tag="nmr")
                nc.vector.tensor_scalar(nmr[:], mv[:, 0:1], rstd[:], -1.0,
                                        ALU.mult, ALU.mult)
                nc.scalar.activation(xt[:], x32, AF.Identity, bias=nmr[:],
                                     scale=rstd[:])
                for j in range(2):
                    pt = pps_t.tile([P, 512], F32R, tag="pt")
                    for jj in range(4):
                        hc = j * 4 + jj
                        nc.tensor.transpose(pt[:, jj * P:(jj + 1) * P],
                                            xt[:, hc * P:(hc + 1) * P], idr[:])
                    nc.scalar.copy(
                        dstT[:, j * 4:(j + 1) * 4, m * P:(m + 1) * P],
                        pt[:].rearrange("p (jj c) -> p jj c", c=P))

        def proj_piece(wtiles, srcT, dstT, bias_sb, m8, n2):
            # one 512-col half of dstT[:, m8, :] = (W.T @ src^T) + bias
            # (bf16, DVE evict - ScalarE must stay free for the exp stream)
            pm = pps_m.tile([P, 512], F32, tag="pm")
            for kk in range(8):
                nc.tensor.matmul(
                    pm[:], wtiles[kk][:, m8 * P:(m8 + 1) * P],
                    srcT[:, kk, n2 * 512:(n2 + 1) * 512],
                    start=(kk == 0), stop=(kk == 7))
            nc.vector.tensor_scalar(
                dstT[:, m8, n2 * 512:(n2 + 1) * 512], pm[:],
                bias_sb[:, m8:m8 + 1], None, ALU.add)

        # ---- LN both inputs
        ctxT = pbig.tile([P, 8, S], BF16, tag="big")
        wv_t = load_w(wv)
        wk_t = load_w(wk)
        wq_t = load_w(wq)
        ln_transpose(cx, ctxT)
        hsT = pbig.tile([P, 8, S], BF16, tag="big")
        ln_transpose(hs, hsT)

        # ---- V projection
        for c in range(8):
            pm = pps_m.tile([P, 1024], F32, tag="pm")
            for n2 in range(2):
                for kk in range(8):
                    nc.tensor.matmul(
                        pm[:, n2 * 512:(n2 + 1) * 512],
                        ctxT[:, kk, c * P:(c + 1) * P],
                        wv_t[kk][:, n2 * 512:(n2 + 1) * 512],
                        start=(kk == 0), stop=(kk == 7))
            nc.vector.tensor_tensor(
                vt[:, c, :, 0:HD], pm[:],
                bv_sb[:].rearrange("p (h c) -> p h c", c=HD), ALU.add)

        attn = pat.tile([P, 8, S], F32R, tag="attn")

        # ---- K/Q projections interleaved with attention.
        pT_tiles = {}

        def scores_chunk(h, c):
            r0 = (h % 2) * HD
            hc = h // 2
            pT = pT_tiles[h]
            pm = pps_m.tile([P, 1024], F32, tag="pm")
            for n2 in range(2):
                nc.tensor.matmul(
                    pm[:, n2 * 512:(n2 + 1) * 512],
                    kT[r0:r0 + HD, hc, c * P:(c + 1) * P],
                    qT[r0:r0 + HD, hc, n2 * 512:(n2 + 1) * 512],
                    start=True, stop=True)
            nc.scalar.activation(pT[:, c, :], pm[:], AF.Exp, scale=0.125)

        def pv_m(h, m):
            pT = pT_tiles[h]
            po = pps_o.tile([P, HD + 1], F32, tag="po")
            for c in range(8):
                nc.tensor.matmul(po[:], pT[:, c, m * P:(m + 1) * P],
                                 vt[:, c, h, :],
                                 start=(c == 0), stop=(c == 7))
            rs = pstat.tile([P, 1], F32, tag="rs2")
            nc.vector.reciprocal(rs[:], po[:, HD:HD + 1])
            nc.vector.tensor_scalar(attn[:, m, h * HD:(h + 1) * HD],
                                    po[:, 0:HD], rs[:], None, ALU.mult)

        # prologue: first K/Q chunk
        for n2 in range(2):
            proj_piece(wk_t, ctxT, kT, bk_sb, 0, n2)
        for n2 in range(2):
            proj_piece(wq_t, hsT, qT, bq_sb, 0, n2)
        for m8 in range(8):
            # next iteration's K/Q chunks, spread through the score stream
            # so ScalarE's exp pipeline is never starved by them
            pieces = []
            if m8 + 1 < 8:
                for wt, st_, dt_, bs in ((wk_t, ctxT, kT, bk_sb),
                                         (wq_t, hsT, qT, bq_sb)):
                    for n2 in range(2):
                        pieces.append((wt, st_, dt_, bs, m8 + 1, n2))
            for h in (2 * m8, 2 * m8 + 1):
                pT_tiles[h] = ppt.tile([P, 8, S], BF16, tag="pT",
                                       name=f"pT{h}")
                for c in range(8):
                    scores_chunk(h, c)
                    if h > 0:
                        pv_m(h - 1, c)
                    if c % 4 == 3 and pieces:
                        proj_piece(*pieces.pop(0))

        # ---- adapter + residual (last head's PV folded into the m loop)
        attn_T = ppt.tile([P, 8, S], BF16, tag="pT", name="attnT")
        for n2 in range(2):
            for mi in range(4):
                m = n2 * 4 + mi
                pv_m(NH - 1, m)
                for j in range(2):
                    pt = pps_t.tile([P, 512], F32R, tag="pt")
                    for jj in range(4):
                        hc2 = j * 4 + jj
                        nc.tensor.transpose(
                            pt[:, jj * P:(jj + 1) * P],
                            attn[:, m, hc2 * P:(hc2 + 1) * P], idr[:])
                    nc.scalar.copy(
                        attn_T[:, j * 4:(j + 1) * 4, m * P:(m + 1) * P],
                        pt[:].rearrange("p (jj c) -> p jj c", c=P))
            pd = pps_m.tile([P, 1024], F32, tag="pm")
            for kk in range(8):
                nc.tensor.matmul(pd[0:HD, 0:512], wd_sb[:, kk, :],
                                 attn_T[:, kk, n2 * 512:(n2 + 1) * 512],
                                 start=(kk == 0), stop=(kk == 7))
            nc.scalar.activation(gt[0:HD, n2 * 512:(n2 + 1) * 512],
                                 pd[0:HD, 0:512], AF.Gelu_apprx_tanh,
                                 bias=bd_sb[:])
            for mi in range(4):
                m = n2 * 4 + mi
                pu = pps_m.tile([P, 1024], F32, tag="pm")
                for nH in range(2):
                    nc.tensor.matmul(pu[:, nH * 512:(nH + 1) * 512],
                                     gt[:, m * P:(m + 1) * P],
                                     wub_sb[:, nH * 512:(nH + 1) * 512],
                                     start=True, stop=True)
                    ot = pout.tile([P, 512], F32, tag="out")
                    nc.vector.tensor_tensor(
                        ot[:], pu[:, nH * 512:(nH + 1) * 512],
                        attn[:, m, nH * 512:(nH + 1) * 512].bitcast(F32),
                        ALU.add)
                    nc.sync.dma_start(
                        out[m * P:(m + 1) * P, nH * 512:(nH + 1) * 512],
                        ot[:])

    nc.compile()
    return nc


def make_in_maps(hidden_states, context, Wq, bq, Wk, bk, Wv, bv,
                 q_gamma, q_beta, c_gamma, c_beta, Wd, bd, Wu, bu):
    f32 = np.float32
    bf = ml_dtypes.bfloat16
    # fold LN gamma/beta into the projection weights (host-side)
    wq_e = (q_gamma[:, None] * Wq).astype(bf)
    bq_e = (bq + q_beta @ Wq).astype(f32)
    wk_e = (c_gamma[:, None] * Wk).astype(bf)
    bk_e = (bk + c_beta @ Wk).astype(f32)
    wv_e = (c_gamma[:, None] * Wv).astype(bf)
    bv_e = (bv + c_beta @ Wv).astype(f32)

    bq_r = np.ascontiguousarray(bq_e.reshape(8, P).T)   # [P, 8]
    bk_r = np.ascontiguousarray(bk_e.reshape(8, P).T)
    bv_r = bv_e.reshape(1, H).astype(bf)
    wd_b = Wd.astype(bf)
    bd_r = bd.reshape(HD, 1).astype(f32)
    wub = np.vstack([Wu, bu.reshape(1, H)]).astype(bf)

    shared = {
        "wq": np.ascontiguousarray(wq_e), "wk": np.ascontiguousarray(wk_e),
        "wv": np.ascontiguousarray(wv_e),
        "bq": bq_r, "bk": bk_r, "bv": bv_r,
        "wd": wd_b, "bd": bd_r, "wub": wub,
    }
    in_maps = []
    for b_ in range(NCORES):
        m = dict(shared)
        m["hs"] = np.ascontiguousarray(hidden_states[b_]).astype(f32)
        m["cx"] = np.ascontiguousarray(context[b_]).astype(f32)
        in_maps.append(m)
    return in_maps


_CACHE = {}


def get_program(reps=1):
    if reps not in _CACHE:
        _CACHE[reps] = build_program(reps=reps)
    return _CACHE[reps]


def kernel(**inputs):
    nc = get_program()
    in_maps = make_in_maps(**{k: np.asarray(v) for k, v in inputs.items()})
    res = run_bass_kernel_spmd(nc, in_maps, list(range(NCORES)))
    out = np.stack([res.results[c]["out"] for c in range(NCORES)], axis=0)
    return out.astype(np.float32)
